# revision 5
# baseline (speedup 1.0000x reference)
"""Bass/Trainium2 kernel for nn_BitwiseBasicBlock.

Computes (reference semantics, NCHW):
    out1 = BN(conv3x3(sign(x), sign(w1)*alpha1), g1, b1)     # training-mode BN
    out2 = BN(conv3x3(sign(out1), sign(w2)*alpha2), g2, b2)
    out  = out2 + x

Strategy:
  - Data-parallel over batch: 32 images -> 8 cores x 4 images. Weights replicated.
  - Sync-BN: per-core per-channel (sum, sumsq) of the integer conv outputs are
    AllReduce'd (2KB payload) so BN stats match the full-batch reference.
  - The conv operands are all +-1, so the 3x3 conv is 9 accumulated matmuls over
    a zero-padded 58-wide activation layout, run in fp8 (exact for +-1) with
    DoubleRow (contracts both 128-channel halves per instruction), accumulating
    exact integers in fp32 PSUM.
  - Engine balance: scalar does binarize + oc0 psum drains, vector does oc1
    drains + bn_stats + incremental stat aggregation, gpsimd does the f16
    x-cache copies (sign reads the f16 cache, decoupling the DMA ring from
    the scalar queue). A dummy warmup AllReduce at t=0 absorbs the
    collective's one-time setup; x DMAs are 4 big transfers per image; the
    finalize tail is pipelined across scalar/vector/gpsimd + DMA.
"""

import os
import sys

import numpy as np

for _p in ("/opt/trn_rl_repo",):
    if _p not in sys.path and os.path.isdir(_p):
        sys.path.insert(0, _p)

import ml_dtypes
from contextlib import ExitStack

import concourse.bass as bass
import concourse.tile as tile
from concourse import bacc
from concourse import mybir
from concourse.bass_utils import run_bass_kernel_spmd

F32 = mybir.dt.float32
F16 = mybir.dt.float16
BF16 = mybir.dt.bfloat16
F8 = mybir.dt.float8e4
F8NP = ml_dtypes.float8_e4m3

EPS = 1e-5
H = W = 56
PW = H + 2            # padded row width
RPT = 8               # output rows per psum tile
NYC = H // RPT        # 7 row-chunks
NT = RPT * PW         # 464 <= 512 (one PSUM bank), incl 16 wrap cols
NS = RPT * W          # 448 packed (clean) elems per row-chunk
SROW = H * W          # 3136 per image-half
CH = 128              # channel chunk (partition dim)
PLANE_F = 3488        # per-half padded plane size; >= 59 + 56*58 + tap slack
OFFS = [(dy, dx) for dy in range(3) for dx in range(3)]
QROWS = ((0, 32), (32, 56))   # x DMA chunks (row ranges, yc-aligned)

USE_COLLECTIVE = True  # AllReduce for BN stats (False: local copy, 1-core only)
PER_SHARD_BN2 = False  # True: skip AR for BN2 (per-shard stats, ~1.3e-2 rel err)

N_CORES = 8
N_IMG = 4             # images per core on HW

GRP = 2               # tail pieces per image-half
GRP_ROWS = H // GRP   # 28 rows per piece


def build_nc(n_img, n_cores):
    nc = bacc.Bacc("TRN2", target_bir_lowering=False)
    x_in = nc.dram_tensor("x", [n_img, 2 * CH, H, W], F32, kind="ExternalInput")
    w1p = nc.dram_tensor("w1p", [CH, 9, 2, 2, CH], F8, kind="ExternalInput")
    w2p = nc.dram_tensor("w2p", [CH, 9, 2, 2, CH], F8, kind="ExternalInput")
    # aux cols (pairs oc0,oc1): P=alpha^2, Q=alpha*g, beta  x  2 convs
    aux = nc.dram_tensor("aux", [CH, 12], F32, kind="ExternalInput")
    out_t = nc.dram_tensor("out", [n_img, 2 * CH, H, W], F32, kind="ExternalOutput")

    count_g = float(n_img * n_cores * SROW)   # global per-channel sample count
    count_l = float(n_img * SROW)             # per-shard count

    with ExitStack() as ctx:
        tc = ctx.enter_context(tile.TileContext(nc))
        singles = ctx.enter_context(tc.tile_pool(name="singles", bufs=1))
        xpool = ctx.enter_context(tc.tile_pool(name="xpool", bufs=3))
        spool = ctx.enter_context(tc.tile_pool(name="spool", bufs=2 * n_img + 2))
        statsp = ctx.enter_context(tc.tile_pool(name="statsp", bufs=1))
        coefp = ctx.enter_context(tc.tile_pool(name="coefp", bufs=1))
        psum = ctx.enter_context(tc.tile_pool(name="psum", bufs=8, space="PSUM"))
        fpool = ctx.enter_context(tc.tile_pool(name="fpool", bufs=4))
        dramp = ctx.enter_context(tc.tile_pool(name="dramp", bufs=1, space="DRAM"))

        w1t = singles.tile([CH, 9, 2, 2, CH], F8)
        nc.sync.dma_start(out=w1t[:], in_=w1p[:])
        w2t = singles.tile([CH, 9, 2, 2, CH], F8)
        nc.sync.dma_start(out=w2t[:], in_=w2p[:])
        auxt = singles.tile([CH, 12], F32)
        nc.sync.dma_start(out=auxt[:], in_=aux[:])
        epst = singles.tile([CH, 1], F32)
        nc.vector.memset(epst[:], EPS)

        # --- plane buffers: 4 dedicated; only pads get zeroed (once) ---------
        planes = [
            singles.tile([CH, 2, PLANE_F], F8, tag=f"plane{n}", name=f"plane{n}")
            for n in range(n_img)
        ]
        for n in range(n_img):
            for j in range(2):
                h = planes[n][:, j]
                nc.vector.memset(h[0:CH, 0:59], 0)                      # head pad
                pairs = h[:, 115 : 115 + (H - 1) * PW].rearrange(
                    "p (y x) -> p y x", x=PW
                )[:, :, 0:2]                                            # row pads
                nc.vector.memset(pairs, 0)
                nc.vector.memset(h[0:CH, 59 + H * PW - 2 : PLANE_F], 0)  # tail pad

        # --- collective warmup (absorbs one-time CC setup latency) ----------
        wi = dramp.tile([CH, 1], F32, tag="wi", name="wi")
        wo = dramp.tile([CH, 1], F32, tag="wo", name="wo")
        nc.sync.dma_start(out=wi[:], in_=epst[:])
        if USE_COLLECTIVE:
            nc.gpsimd.collective_compute(
                "AllReduce",
                mybir.AluOpType.add,
                replica_groups=[list(range(n_cores))],
                ins=[wi[:].opt()],
                outs=[wo[:].opt()],
            )

        # --- x cached in SBUF as f16; sign reads this (exact: no x value
        # flushes to zero in f16 for N(0,1) fp32 data above 6e-8) ------------
        x16 = {
            (n, j): singles.tile([CH, SROW], F16, tag=f"x16_{n}_{j}", name=f"x16_{n}_{j}")
            for n in range(n_img)
            for j in range(2)
        }

        def load_x(n):
            """DMA x image n (4 transfers), convert to f16 cache on gpsimd."""
            for r0, r1 in QROWS:
                for j in range(2):
                    xs = xpool.tile([CH, (r1 - r0) * W], F32, tag="xs", name="xs")
                    nc.sync.dma_start(
                        out=xs[:],
                        in_=x_in[
                            n, j * CH : (j + 1) * CH, r0:r1, :
                        ].rearrange("c h w -> c (h w)"),
                    )
                    nc.gpsimd.tensor_scalar_mul(
                        x16[(n, j)][:, r0 * W : r1 * W], xs[:], 1.0
                    )

        def sign_x(n, row_chunks):
            """plane[n] <- sign(x16[n]) for given row ranges (scalar)."""
            for r0, r1 in row_chunks:
                for j in range(2):
                    dst = (
                        planes[n][:, j, 59 + r0 * PW : 59 + r1 * PW]
                        .rearrange("p (y x) -> p y x", x=PW)[:, :, 0:W]
                    )
                    nc.scalar.activation(
                        out=dst,
                        in_=x16[(n, j)][:, r0 * W : r1 * W].rearrange(
                            "p (y x) -> p y x", x=W
                        ),
                        func=mybir.ActivationFunctionType.Sign,
                    )

        def conv_tile(wt, plane, oc, yc, ps):
            """9 accumulated fp8 DoubleRow matmuls into psum tile ps."""
            for k in range(9):
                dy, dx = OFFS[k]
                off = yc * RPT * PW + dy * PW + dx
                nc.tensor.matmul(
                    out=ps[:],
                    lhsT=wt[:, k, oc],
                    rhs=plane[:, :, off : off + NT],
                    start=(k == 0),
                    stop=(k == 8),
                    perf_mode=mybir.MatmulPerfMode.DoubleRow,
                )

        def conv_image(wt, n, layer, sdict, acc):
            """Conv both oc halves of image n; drain psum into packed f16 s
            (oc0 on scalar, oc1 on vector); bn_stats per tile on vector;
            incremental (sum, sumsq) accumulate into acc
            (cols 0:2 sums oc0,oc1; cols 2:4 sumsqs)."""
            for oc in range(2):
                s = spool.tile([CH, NYC, NS], F16, tag="simg", name="simg")
                sdict[(n, oc)] = s
                bnb = statsp.tile(
                    [CH, NYC, 6], F32, tag=f"bnb{layer}_{n}_{oc}", name=f"bnb{layer}_{n}_{oc}"
                )
                for yc in range(NYC):
                    ps = psum.tile([CH, NT], F32, tag="ps", name="ps")
                    conv_tile(wt, planes[n], oc, yc, ps)
                    psv = ps[:].rearrange("p (r x) -> p r x", x=PW)[:, :, 0:W]
                    sv = s[:, yc].rearrange("p (r x) -> p r x", x=W)
                    if oc == 0:
                        nc.scalar.activation(
                            out=sv, in_=psv,
                            func=mybir.ActivationFunctionType.Copy,
                        )
                    else:
                        nc.vector.tensor_scalar_mul(sv, psv, 1.0)
                    nc.vector.bn_stats(out=bnb[:, yc], in_=s[:, yc])
                mv = coefp.tile([CH, 2], F32, tag=f"mv{layer}{oc}", name=f"mv{layer}{oc}")
                nc.vector.bn_aggr(out=mv[:], in_=bnb[:].rearrange("p a s -> p (a s)"))
                ns = float(NYC * NS)
                so, qo = oc, 2 + oc
                if n == 0:
                    nc.vector.tensor_scalar_mul(acc[:, so : so + 1], mv[:, 0:1], ns)
                    nc.vector.tensor_mul(acc[:, qo : qo + 1], mv[:, 0:1], mv[:, 0:1])
                    nc.vector.tensor_add(acc[:, qo : qo + 1], acc[:, qo : qo + 1], mv[:, 1:2])
                    nc.vector.tensor_scalar_mul(acc[:, qo : qo + 1], acc[:, qo : qo + 1], ns)
                else:
                    t = coefp.tile([CH, 2], F32, tag=f"t{layer}{oc}", name=f"t{layer}{oc}")
                    nc.vector.tensor_scalar_mul(t[:, 0:1], mv[:, 0:1], ns)
                    nc.vector.tensor_mul(t[:, 1:2], mv[:, 0:1], mv[:, 0:1])
                    nc.vector.tensor_add(t[:, 1:2], t[:, 1:2], mv[:, 1:2])
                    nc.vector.tensor_scalar_mul(t[:, 1:2], t[:, 1:2], ns)
                    nc.vector.tensor_add(acc[:, so : so + 1], acc[:, so : so + 1], t[:, 0:1])
                    nc.vector.tensor_add(acc[:, qo : qo + 1], acc[:, qo : qo + 1], t[:, 1:2])

        def make_coefs(cc, layer, count):
            """cc = per-channel [sum0, sum1, sq0, sq1] -> A,B with
            BN(alpha*S)*g + b == S*A + B (both oc as [CH,2] columns)."""
            base = 6 * (layer - 1)
            P = auxt[:, base + 0 : base + 2]
            Q = auxt[:, base + 2 : base + 4]
            beta = auxt[:, base + 4 : base + 6]
            m = coefp.tile([CH, 2], F32, tag=f"m{layer}", name=f"m{layer}")
            nc.vector.tensor_scalar_mul(m[:], cc[:, 0:2], 1.0 / count)
            v = coefp.tile([CH, 2], F32, tag=f"v{layer}", name=f"v{layer}")
            nc.vector.tensor_scalar_mul(v[:], cc[:, 2:4], 1.0 / count)
            mm = coefp.tile([CH, 2], F32, tag=f"mm{layer}", name=f"mm{layer}")
            nc.vector.tensor_mul(mm[:], m[:], m[:])
            nc.vector.tensor_sub(v[:], v[:], mm[:])       # var of S
            nc.vector.tensor_mul(v[:], v[:], P)           # var of alpha*S
            sd = coefp.tile([CH, 2], F32, tag=f"sd{layer}", name=f"sd{layer}")
            nc.scalar.activation(
                out=sd[:], in_=v[:],
                func=mybir.ActivationFunctionType.Sqrt,
                bias=epst[:], scale=1.0,
            )
            r = coefp.tile([CH, 2], F32, tag=f"r{layer}", name=f"r{layer}")
            nc.vector.reciprocal(r[:], sd[:])
            A = coefp.tile([CH, 2], F32, tag=f"A{layer}", name=f"A{layer}")
            nc.vector.tensor_mul(A[:], Q, r[:])
            B = coefp.tile([CH, 2], F32, tag=f"B{layer}", name=f"B{layer}")
            nc.vector.tensor_mul(B[:], m[:], A[:])
            nc.vector.tensor_sub(B[:], beta, B[:])
            return A, B

        def allreduce_stats(acc, layer):
            cci = dramp.tile([CH, 4], F32, tag=f"cci{layer}", name=f"cci{layer}")
            cco = dramp.tile([CH, 4], F32, tag=f"cco{layer}", name=f"cco{layer}")
            nc.sync.dma_start(out=cci[:], in_=acc[:])
            if USE_COLLECTIVE:
                nc.gpsimd.collective_compute(
                    "AllReduce",
                    mybir.AluOpType.add,
                    replica_groups=[list(range(n_cores))],
                    ins=[cci[:].opt()],
                    outs=[cco[:].opt()],
                )
            else:
                nc.sync.dma_start(out=cco[:], in_=cci[:])
            ccg = coefp.tile([CH, 4], F32, tag=f"ccg{layer}", name=f"ccg{layer}")
            nc.sync.dma_start(out=ccg[:], in_=cco[:])
            return ccg

        def binarize2(n, A1, B1, row_chunks):
            """plane[n] <- sign(A1*s1 + B1), rows chunked for overlap."""
            for r0, r1 in row_chunks:
                for j in range(2):
                    s = s1[(n, j)]
                    flat = s[:].rearrange("p y t -> p (y t)")
                    base = planes[n][:, j, 59 : 59 + H * PW].rearrange(
                        "p (y x) -> p y x", x=PW
                    )
                    nc.scalar.activation(
                        out=base[:, r0:r1, 0:W],
                        in_=flat[:, r0 * W : r1 * W].rearrange("p (y x) -> p y x", x=W),
                        func=mybir.ActivationFunctionType.Sign,
                        scale=A1[:, j : j + 1],
                        bias=B1[:, j : j + 1],
                    )

        # ================= layer 1 =================
        acc1 = coefp.tile([CH, 4], F32, tag="acc1", name="acc1")
        s1 = {}
        load_x(0)
        sign_x(0, [(0, 16), (16, 32), (32, 56)])
        load_x(1)
        sign_x(1, [QROWS[0], QROWS[1]])
        conv_image(w1t, 0, 1, s1, acc1)
        load_x(2)
        sign_x(2, [QROWS[0], QROWS[1]])
        conv_image(w1t, 1, 1, s1, acc1)
        load_x(3)
        sign_x(3, [QROWS[0], QROWS[1]])
        conv_image(w1t, 2, 1, s1, acc1)
        conv_image(w1t, 3, 1, s1, acc1)

        ccg1 = allreduce_stats(acc1, 1)
        A1, B1 = make_coefs(ccg1, 1, count_g)

        # ================= binarize + layer 2 =================
        acc2 = coefp.tile([CH, 4], F32, tag="acc2", name="acc2")
        s2 = {}
        binarize2(0, A1, B1, [(0, 16), (16, 32), (32, 56)])
        binarize2(1, A1, B1, [(0, 56)])
        conv_image(w2t, 0, 2, s2, acc2)
        binarize2(2, A1, B1, [(0, 56)])
        conv_image(w2t, 1, 2, s2, acc2)
        binarize2(3, A1, B1, [(0, 56)])
        conv_image(w2t, 2, 2, s2, acc2)
        conv_image(w2t, 3, 2, s2, acc2)

        if PER_SHARD_BN2:
            A2, B2 = make_coefs(acc2, 2, count_l)
        else:
            ccg2 = allreduce_stats(acc2, 2)
            A2, B2 = make_coefs(ccg2, 2, count_g)

        # ================= tail: affine + residual + store =================
        # oc0 chain: scalar affine -> vector add; oc1: vector affine -> gpsimd add
        for n in range(n_img):
            for oc in range(2):
                s = s2[(n, oc)]
                flat = s[:].rearrange("p y t -> p (y t)")
                for g in range(GRP):
                    e0 = g * GRP_ROWS * W
                    e1 = (g + 1) * GRP_ROWS * W
                    fin = fpool.tile([CH, GRP_ROWS * W], F32, tag="fin", name="fin")
                    if oc == 0:
                        nc.scalar.activation(
                            out=fin[:],
                            in_=flat[:, e0:e1],
                            func=mybir.ActivationFunctionType.Identity,
                            scale=A2[:, oc : oc + 1],
                            bias=B2[:, oc : oc + 1],
                        )
                        nc.vector.tensor_add(fin[:], fin[:], x16[(n, oc)][:, e0:e1])
                    else:
                        nc.vector.tensor_scalar(
                            fin[:], flat[:, e0:e1],
                            A2[:, oc : oc + 1], B2[:, oc : oc + 1],
                            mybir.AluOpType.mult, mybir.AluOpType.add,
                        )
                        nc.gpsimd.tensor_add(fin[:], fin[:], x16[(n, oc)][:, e0:e1])
                    nc.sync.dma_start(
                        out=out_t[
                            n, oc * CH : (oc + 1) * CH,
                            g * GRP_ROWS : (g + 1) * GRP_ROWS, :,
                        ],
                        in_=fin[:].rearrange("p (y x) -> p y x", x=W),
                    )

    if not nc.is_finalized():
        nc.finalize()
    return nc


def pack_weights(w):
    """w [256,256,3,3] f32 -> [128(c), 9(off), 2(oc), 2(j), 128(o)] sign in fp8."""
    s = np.sign(w).astype(np.float32)          # [O, I, 3, 3]
    s = s.reshape(2, CH, 2, CH, 3, 3)          # [oc, o, j, c, dy, dx]
    s = s.transpose(3, 4, 5, 0, 2, 1)          # [c, dy, dx, oc, j, o]
    s = np.ascontiguousarray(s.reshape(CH, 9, 2, 2, CH))
    return s.astype(F8NP)


def pack_aux(w1, g1, b1, w2, g2, b2):
    aux = np.zeros((CH, 12), np.float32)
    for conv, (w, g, b) in enumerate(((w1, g1, b1), (w2, g2, b2))):
        alpha = np.abs(w).mean(axis=(1, 2, 3), dtype=np.float32)  # [256]
        base = 6 * conv
        for oc in range(2):
            sl = slice(oc * CH, (oc + 1) * CH)
            aux[:, base + 0 + oc] = alpha[sl] * alpha[sl]
            aux[:, base + 2 + oc] = alpha[sl] * g[sl]
            aux[:, base + 4 + oc] = b[sl]
    return aux


_NC_CACHE = {}


def _ensure_ntff_hook():
    """Register the axon NTFF profiling hook if the image's antenv lacks it."""
    import types

    try:
        from antenv.axon_hooks import get_axon_ntff_profile_hook  # noqa: F401
        return
    except ImportError:
        pass
    try:
        import antenv
        from trn_agent_boot.trn_boot import _ntff_profile_via_ctypes

        hook = _ntff_profile_via_ctypes("/opt/axon/libaxon_pjrt.so")
        mod = types.ModuleType("antenv.axon_hooks")
        mod._hook = hook

        def set_axon_ntff_profile_hook(h):
            mod._hook = h

        def get_axon_ntff_profile_hook():
            return mod._hook

        mod.set_axon_ntff_profile_hook = set_axon_ntff_profile_hook
        mod.get_axon_ntff_profile_hook = get_axon_ntff_profile_hook
        sys.modules["antenv.axon_hooks"] = mod
        antenv.axon_hooks = mod
    except Exception:
        pass


def kernel(x, w1, g1, b1, w2, g2, b2, _trace=False):
    x = np.asarray(x, np.float32)
    n_total = x.shape[0]
    assert n_total == N_CORES * N_IMG, x.shape
    key = (N_IMG, N_CORES)
    if key not in _NC_CACHE:
        _NC_CACHE[key] = build_nc(N_IMG, N_CORES)
    nc = _NC_CACHE[key]

    w1p = pack_weights(np.asarray(w1, np.float32))
    w2p = pack_weights(np.asarray(w2, np.float32))
    aux = pack_aux(
        np.asarray(w1, np.float32), np.asarray(g1, np.float32), np.asarray(b1, np.float32),
        np.asarray(w2, np.float32), np.asarray(g2, np.float32), np.asarray(b2, np.float32),
    )

    if _trace:
        _ensure_ntff_hook()
    in_maps = [
        {
            "x": np.ascontiguousarray(x[c * N_IMG : (c + 1) * N_IMG]),
            "w1p": w1p,
            "w2p": w2p,
            "aux": aux,
        }
        for c in range(N_CORES)
    ]
    res = run_bass_kernel_spmd(
        nc, in_maps, core_ids=list(range(N_CORES)), trace=_trace
    )
    out = np.concatenate([r["out"] for r in res.results], axis=0).astype(np.float32)
    if _trace:
        return out, res
    return out


# revision 7
# speedup vs baseline: 1.8707x; 1.8707x over previous
"""Bass/Trainium2 kernel for nn_BitwiseBasicBlock.

Computes (reference semantics, NCHW):
    out1 = BN(conv3x3(sign(x), sign(w1)*alpha1), g1, b1)     # training-mode BN
    out2 = BN(conv3x3(sign(out1), sign(w2)*alpha2), g2, b2)
    out  = out2 + x

Strategy:
  - Data-parallel over batch: 32 images -> 8 cores x 4 images. Weights replicated.
  - Sync-BN: per-core per-channel (sum, sumsq) of the integer conv outputs are
    AllReduce'd (2KB payload) so BN stats match the full-batch reference.
  - The conv operands are all +-1, so the 3x3 conv is 9 accumulated matmuls over
    a zero-padded 58-wide activation layout, run in fp8 (exact for +-1) with
    DoubleRow (contracts both 128-channel halves per instruction), accumulating
    exact integers in fp32 PSUM.
  - Engine balance: scalar does binarize + oc0 psum drains, vector does oc1
    drains + bn_stats + incremental stat aggregation, gpsimd does the f16
    x-cache copies (sign reads the f16 cache, decoupling the DMA ring from
    the scalar queue). A dummy warmup AllReduce at t=0 absorbs the
    collective's one-time setup; x DMAs are 4 big transfers per image; the
    finalize tail is pipelined across scalar/vector/gpsimd + DMA.
"""

import os
import sys

import numpy as np

for _p in ("/opt/trn_rl_repo",):
    if _p not in sys.path and os.path.isdir(_p):
        sys.path.insert(0, _p)

import ml_dtypes
from contextlib import ExitStack

import concourse.bass as bass
import concourse.tile as tile
from concourse import bacc
from concourse import mybir
from concourse.bass_utils import run_bass_kernel_spmd

F32 = mybir.dt.float32
F16 = mybir.dt.float16
BF16 = mybir.dt.bfloat16
F8 = mybir.dt.float8e4
F8NP = ml_dtypes.float8_e4m3

EPS = 1e-5
H = W = 56
PW = H + 2            # padded row width
RPT = 8               # output rows per psum tile
NYC = H // RPT        # 7 row-chunks
NT = RPT * PW         # 464 <= 512 (one PSUM bank), incl 16 wrap cols
NS = RPT * W          # 448 packed (clean) elems per row-chunk
SROW = H * W          # 3136 per image-half
CH = 128              # channel chunk (partition dim)
PLANE_F = 3488        # per-half padded plane size; >= 59 + 56*58 + tap slack
OFFS = [(dy, dx) for dy in range(3) for dx in range(3)]
QROWS = ((0, 32), (32, 56))   # x DMA chunks (row ranges, yc-aligned)

USE_COLLECTIVE = True  # AllReduce for BN stats (False: local copy, 1-core only)
PER_SHARD_BN2 = False  # True: skip AR for BN2 (per-shard stats, ~1.3e-2 rel err)

N_CORES = 8
N_IMG = 4             # images per core on HW

GRP = 2               # tail pieces per image-half
GRP_ROWS = H // GRP   # 28 rows per piece


def build_nc(n_img, n_cores):
    nc = bacc.Bacc("TRN2", target_bir_lowering=False)
    x_in = nc.dram_tensor("x", [n_img, 2 * CH, H, W], F32, kind="ExternalInput")
    w1p = nc.dram_tensor("w1p", [CH, 9, 2, 2, CH], F8, kind="ExternalInput")
    w2p = nc.dram_tensor("w2p", [CH, 9, 2, 2, CH], F8, kind="ExternalInput")
    # aux cols (pairs oc0,oc1): P=alpha^2, Q=alpha*g, beta  x  2 convs
    aux = nc.dram_tensor("aux", [CH, 12], F32, kind="ExternalInput")
    out_t = nc.dram_tensor("out", [n_img, 2 * CH, H, W], F32, kind="ExternalOutput")

    count_g = float(n_img * n_cores * SROW)   # global per-channel sample count
    count_l = float(n_img * SROW)             # per-shard count

    with ExitStack() as ctx:
        tc = ctx.enter_context(tile.TileContext(nc))
        singles = ctx.enter_context(tc.tile_pool(name="singles", bufs=1))
        xpool = ctx.enter_context(tc.tile_pool(name="xpool", bufs=3))
        spool = ctx.enter_context(tc.tile_pool(name="spool", bufs=2 * n_img + 2))
        statsp = ctx.enter_context(tc.tile_pool(name="statsp", bufs=1))
        coefp = ctx.enter_context(tc.tile_pool(name="coefp", bufs=1))
        psum = ctx.enter_context(tc.tile_pool(name="psum", bufs=8, space="PSUM"))
        fpool = ctx.enter_context(tc.tile_pool(name="fpool", bufs=4))
        dramp = ctx.enter_context(tc.tile_pool(name="dramp", bufs=1, space="DRAM"))

        w1t = singles.tile([CH, 9, 2, 2, CH], F8)
        nc.sync.dma_start(out=w1t[:], in_=w1p[:])
        w2t = singles.tile([CH, 9, 2, 2, CH], F8)
        nc.sync.dma_start(out=w2t[:], in_=w2p[:])
        auxt = singles.tile([CH, 12], F32)
        nc.sync.dma_start(out=auxt[:], in_=aux[:])
        epst = singles.tile([CH, 1], F32)
        nc.vector.memset(epst[:], EPS)

        # --- plane buffers: 4 dedicated; only pads get zeroed (once) ---------
        planes = [
            singles.tile([CH, 2, PLANE_F], F8, tag=f"plane{n}", name=f"plane{n}")
            for n in range(n_img)
        ]
        for n in range(n_img):
            for j in range(2):
                h = planes[n][:, j]
                nc.vector.memset(h[0:CH, 0:59], 0)                      # head pad
                pairs = h[:, 115 : 115 + (H - 1) * PW].rearrange(
                    "p (y x) -> p y x", x=PW
                )[:, :, 0:2]                                            # row pads
                nc.vector.memset(pairs, 0)
                nc.vector.memset(h[0:CH, 59 + H * PW - 2 : PLANE_F], 0)  # tail pad

        # --- collective warmup (absorbs one-time CC setup latency) ----------
        wi = dramp.tile([CH, 1], F32, tag="wi", name="wi")
        wo = dramp.tile([CH, 1], F32, tag="wo", name="wo")
        nc.sync.dma_start(out=wi[:], in_=epst[:])
        if USE_COLLECTIVE:
            nc.gpsimd.collective_compute(
                "AllReduce",
                mybir.AluOpType.add,
                replica_groups=[list(range(n_cores))],
                ins=[wi[:].opt()],
                outs=[wo[:].opt()],
            )

        # --- x cached in SBUF as f16; sign reads this (exact: no x value
        # flushes to zero in f16 for N(0,1) fp32 data above 6e-8) ------------
        x16 = {
            (n, j): singles.tile([CH, SROW], F16, tag=f"x16_{n}_{j}", name=f"x16_{n}_{j}")
            for n in range(n_img)
            for j in range(2)
        }

        def load_x(n):
            """DMA x image n (4 transfers), convert to f16 cache on gpsimd."""
            for r0, r1 in QROWS:
                for j in range(2):
                    xs = xpool.tile([CH, (r1 - r0) * W], F32, tag="xs", name="xs")
                    nc.sync.dma_start(
                        out=xs[:],
                        in_=x_in[
                            n, j * CH : (j + 1) * CH, r0:r1, :
                        ].rearrange("c h w -> c (h w)"),
                    )
                    nc.vector.tensor_scalar_mul(
                        x16[(n, j)][:, r0 * W : r1 * W], xs[:], 1.0
                    )

        def sign_x(n, row_chunks):
            """plane[n] <- sign(x16[n]) for given row ranges (scalar)."""
            for r0, r1 in row_chunks:
                for j in range(2):
                    dst = (
                        planes[n][:, j, 59 + r0 * PW : 59 + r1 * PW]
                        .rearrange("p (y x) -> p y x", x=PW)[:, :, 0:W]
                    )
                    nc.scalar.activation(
                        out=dst,
                        in_=x16[(n, j)][:, r0 * W : r1 * W].rearrange(
                            "p (y x) -> p y x", x=W
                        ),
                        func=mybir.ActivationFunctionType.Sign,
                    )

        def conv_tile(wt, plane, oc, yc, ps):
            """9 accumulated fp8 DoubleRow matmuls into psum tile ps."""
            for k in range(9):
                dy, dx = OFFS[k]
                off = yc * RPT * PW + dy * PW + dx
                nc.tensor.matmul(
                    out=ps[:],
                    lhsT=wt[:, k, oc],
                    rhs=plane[:, :, off : off + NT],
                    start=(k == 0),
                    stop=(k == 8),
                    perf_mode=mybir.MatmulPerfMode.DoubleRow,
                )

        def conv_image(wt, n, layer, sdict, acc):
            """Conv both oc halves of image n; drain psum into packed f16 s
            (oc0 on scalar, oc1 on vector); bn_stats per tile on vector;
            incremental (sum, sumsq) accumulate into acc
            (cols 0:2 sums oc0,oc1; cols 2:4 sumsqs)."""
            for oc in range(2):
                s = spool.tile([CH, NYC, NS], F16, tag="simg", name="simg")
                sdict[(n, oc)] = s
                bnb = statsp.tile(
                    [CH, NYC, 6], F32, tag=f"bnb{layer}_{n}_{oc}", name=f"bnb{layer}_{n}_{oc}"
                )
                for yc in range(NYC):
                    ps = psum.tile([CH, NT], F32, tag="ps", name="ps")
                    conv_tile(wt, planes[n], oc, yc, ps)
                    psv = ps[:].rearrange("p (r x) -> p r x", x=PW)[:, :, 0:W]
                    sv = s[:, yc].rearrange("p (r x) -> p r x", x=W)
                    if oc == 0:
                        nc.scalar.activation(
                            out=sv, in_=psv,
                            func=mybir.ActivationFunctionType.Copy,
                        )
                    else:
                        nc.vector.tensor_scalar_mul(sv, psv, 1.0)
                    nc.vector.bn_stats(out=bnb[:, yc], in_=s[:, yc])
                mv = coefp.tile([CH, 2], F32, tag=f"mv{layer}{oc}", name=f"mv{layer}{oc}")
                nc.vector.bn_aggr(out=mv[:], in_=bnb[:].rearrange("p a s -> p (a s)"))
                ns = float(NYC * NS)
                so, qo = oc, 2 + oc
                if n == 0:
                    nc.vector.tensor_scalar_mul(acc[:, so : so + 1], mv[:, 0:1], ns)
                    nc.vector.tensor_mul(acc[:, qo : qo + 1], mv[:, 0:1], mv[:, 0:1])
                    nc.vector.tensor_add(acc[:, qo : qo + 1], acc[:, qo : qo + 1], mv[:, 1:2])
                    nc.vector.tensor_scalar_mul(acc[:, qo : qo + 1], acc[:, qo : qo + 1], ns)
                else:
                    t = coefp.tile([CH, 2], F32, tag=f"t{layer}{oc}", name=f"t{layer}{oc}")
                    nc.vector.tensor_scalar_mul(t[:, 0:1], mv[:, 0:1], ns)
                    nc.vector.tensor_mul(t[:, 1:2], mv[:, 0:1], mv[:, 0:1])
                    nc.vector.tensor_add(t[:, 1:2], t[:, 1:2], mv[:, 1:2])
                    nc.vector.tensor_scalar_mul(t[:, 1:2], t[:, 1:2], ns)
                    nc.vector.tensor_add(acc[:, so : so + 1], acc[:, so : so + 1], t[:, 0:1])
                    nc.vector.tensor_add(acc[:, qo : qo + 1], acc[:, qo : qo + 1], t[:, 1:2])

        def make_coefs(cc, layer, count):
            """cc = per-channel [sum0, sum1, sq0, sq1] -> A,B with
            BN(alpha*S)*g + b == S*A + B (both oc as [CH,2] columns)."""
            base = 6 * (layer - 1)
            P = auxt[:, base + 0 : base + 2]
            Q = auxt[:, base + 2 : base + 4]
            beta = auxt[:, base + 4 : base + 6]
            m = coefp.tile([CH, 2], F32, tag=f"m{layer}", name=f"m{layer}")
            nc.vector.tensor_scalar_mul(m[:], cc[:, 0:2], 1.0 / count)
            v = coefp.tile([CH, 2], F32, tag=f"v{layer}", name=f"v{layer}")
            nc.vector.tensor_scalar_mul(v[:], cc[:, 2:4], 1.0 / count)
            mm = coefp.tile([CH, 2], F32, tag=f"mm{layer}", name=f"mm{layer}")
            nc.vector.tensor_mul(mm[:], m[:], m[:])
            nc.vector.tensor_sub(v[:], v[:], mm[:])       # var of S
            nc.vector.tensor_mul(v[:], v[:], P)           # var of alpha*S
            sd = coefp.tile([CH, 2], F32, tag=f"sd{layer}", name=f"sd{layer}")
            nc.scalar.activation(
                out=sd[:], in_=v[:],
                func=mybir.ActivationFunctionType.Sqrt,
                bias=epst[:], scale=1.0,
            )
            r = coefp.tile([CH, 2], F32, tag=f"r{layer}", name=f"r{layer}")
            nc.vector.reciprocal(r[:], sd[:])
            A = coefp.tile([CH, 2], F32, tag=f"A{layer}", name=f"A{layer}")
            nc.vector.tensor_mul(A[:], Q, r[:])
            B = coefp.tile([CH, 2], F32, tag=f"B{layer}", name=f"B{layer}")
            nc.vector.tensor_mul(B[:], m[:], A[:])
            nc.vector.tensor_sub(B[:], beta, B[:])
            return A, B

        def allreduce_stats(acc, layer):
            cci = dramp.tile([CH, 4], F32, tag=f"cci{layer}", name=f"cci{layer}")
            cco = dramp.tile([CH, 4], F32, tag=f"cco{layer}", name=f"cco{layer}")
            nc.sync.dma_start(out=cci[:], in_=acc[:])
            if USE_COLLECTIVE:
                nc.gpsimd.collective_compute(
                    "AllReduce",
                    mybir.AluOpType.add,
                    replica_groups=[list(range(n_cores))],
                    ins=[cci[:].opt()],
                    outs=[cco[:].opt()],
                )
            else:
                nc.sync.dma_start(out=cco[:], in_=cci[:])
            ccg = coefp.tile([CH, 4], F32, tag=f"ccg{layer}", name=f"ccg{layer}")
            nc.sync.dma_start(out=ccg[:], in_=cco[:])
            return ccg

        def binarize2(n, A1, B1, row_chunks):
            """plane[n] <- sign(A1*s1 + B1), rows chunked for overlap."""
            for r0, r1 in row_chunks:
                for j in range(2):
                    s = s1[(n, j)]
                    flat = s[:].rearrange("p y t -> p (y t)")
                    base = planes[n][:, j, 59 : 59 + H * PW].rearrange(
                        "p (y x) -> p y x", x=PW
                    )
                    nc.scalar.activation(
                        out=base[:, r0:r1, 0:W],
                        in_=flat[:, r0 * W : r1 * W].rearrange("p (y x) -> p y x", x=W),
                        func=mybir.ActivationFunctionType.Sign,
                        scale=A1[:, j : j + 1],
                        bias=B1[:, j : j + 1],
                    )

        # ================= layer 1 =================
        acc1 = coefp.tile([CH, 4], F32, tag="acc1", name="acc1")
        s1 = {}
        load_x(0)
        sign_x(0, [(0, 16), (16, 32), (32, 56)])
        load_x(1)
        sign_x(1, [QROWS[0], QROWS[1]])
        conv_image(w1t, 0, 1, s1, acc1)
        load_x(2)
        sign_x(2, [QROWS[0], QROWS[1]])
        conv_image(w1t, 1, 1, s1, acc1)
        load_x(3)
        sign_x(3, [QROWS[0], QROWS[1]])
        conv_image(w1t, 2, 1, s1, acc1)
        conv_image(w1t, 3, 1, s1, acc1)

        ccg1 = allreduce_stats(acc1, 1)
        A1, B1 = make_coefs(ccg1, 1, count_g)

        # ================= binarize + layer 2 =================
        acc2 = coefp.tile([CH, 4], F32, tag="acc2", name="acc2")
        s2 = {}
        binarize2(0, A1, B1, [(0, 16), (16, 32), (32, 56)])
        binarize2(1, A1, B1, [(0, 56)])
        conv_image(w2t, 0, 2, s2, acc2)
        binarize2(2, A1, B1, [(0, 56)])
        conv_image(w2t, 1, 2, s2, acc2)
        binarize2(3, A1, B1, [(0, 56)])
        conv_image(w2t, 2, 2, s2, acc2)
        conv_image(w2t, 3, 2, s2, acc2)

        if PER_SHARD_BN2:
            A2, B2 = make_coefs(acc2, 2, count_l)
        else:
            ccg2 = allreduce_stats(acc2, 2)
            A2, B2 = make_coefs(ccg2, 2, count_g)

        # ================= tail: affine + residual + store =================
        # oc0 chain: scalar affine -> vector add; oc1: vector affine -> gpsimd add
        for n in range(n_img):
            for oc in range(2):
                s = s2[(n, oc)]
                flat = s[:].rearrange("p y t -> p (y t)")
                for g in range(GRP):
                    e0 = g * GRP_ROWS * W
                    e1 = (g + 1) * GRP_ROWS * W
                    fin = fpool.tile([CH, GRP_ROWS * W], F32, tag="fin", name="fin")
                    if oc == 0:
                        nc.scalar.activation(
                            out=fin[:],
                            in_=flat[:, e0:e1],
                            func=mybir.ActivationFunctionType.Identity,
                            scale=A2[:, oc : oc + 1],
                            bias=B2[:, oc : oc + 1],
                        )
                    else:
                        nc.vector.tensor_scalar(
                            fin[:], flat[:, e0:e1],
                            A2[:, oc : oc + 1], B2[:, oc : oc + 1],
                            mybir.AluOpType.mult, mybir.AluOpType.add,
                        )
                    nc.vector.tensor_add(fin[:], fin[:], x16[(n, oc)][:, e0:e1])
                    nc.sync.dma_start(
                        out=out_t[
                            n, oc * CH : (oc + 1) * CH,
                            g * GRP_ROWS : (g + 1) * GRP_ROWS, :,
                        ],
                        in_=fin[:].rearrange("p (y x) -> p y x", x=W),
                    )

    if not nc.is_finalized():
        nc.finalize()
    return nc


def pack_weights(w):
    """w [256,256,3,3] f32 -> [128(c), 9(off), 2(oc), 2(j), 128(o)] sign in fp8."""
    s = np.sign(w).astype(np.float32)          # [O, I, 3, 3]
    s = s.reshape(2, CH, 2, CH, 3, 3)          # [oc, o, j, c, dy, dx]
    s = s.transpose(3, 4, 5, 0, 2, 1)          # [c, dy, dx, oc, j, o]
    s = np.ascontiguousarray(s.reshape(CH, 9, 2, 2, CH))
    return s.astype(F8NP)


def pack_aux(w1, g1, b1, w2, g2, b2):
    aux = np.zeros((CH, 12), np.float32)
    for conv, (w, g, b) in enumerate(((w1, g1, b1), (w2, g2, b2))):
        alpha = np.abs(w).mean(axis=(1, 2, 3), dtype=np.float32)  # [256]
        base = 6 * conv
        for oc in range(2):
            sl = slice(oc * CH, (oc + 1) * CH)
            aux[:, base + 0 + oc] = alpha[sl] * alpha[sl]
            aux[:, base + 2 + oc] = alpha[sl] * g[sl]
            aux[:, base + 4 + oc] = b[sl]
    return aux


_NC_CACHE = {}


def _ensure_ntff_hook():
    """Register the axon NTFF profiling hook if the image's antenv lacks it."""
    import types

    try:
        from antenv.axon_hooks import get_axon_ntff_profile_hook  # noqa: F401
        return
    except ImportError:
        pass
    try:
        import antenv
        from trn_agent_boot.trn_boot import _ntff_profile_via_ctypes

        hook = _ntff_profile_via_ctypes("/opt/axon/libaxon_pjrt.so")
        mod = types.ModuleType("antenv.axon_hooks")
        mod._hook = hook

        def set_axon_ntff_profile_hook(h):
            mod._hook = h

        def get_axon_ntff_profile_hook():
            return mod._hook

        mod.set_axon_ntff_profile_hook = set_axon_ntff_profile_hook
        mod.get_axon_ntff_profile_hook = get_axon_ntff_profile_hook
        sys.modules["antenv.axon_hooks"] = mod
        antenv.axon_hooks = mod
    except Exception:
        pass


def kernel(x, w1, g1, b1, w2, g2, b2, _trace=False):
    x = np.asarray(x, np.float32)
    n_total = x.shape[0]
    assert n_total == N_CORES * N_IMG, x.shape
    key = (N_IMG, N_CORES)
    if key not in _NC_CACHE:
        _NC_CACHE[key] = build_nc(N_IMG, N_CORES)
    nc = _NC_CACHE[key]

    w1p = pack_weights(np.asarray(w1, np.float32))
    w2p = pack_weights(np.asarray(w2, np.float32))
    aux = pack_aux(
        np.asarray(w1, np.float32), np.asarray(g1, np.float32), np.asarray(b1, np.float32),
        np.asarray(w2, np.float32), np.asarray(g2, np.float32), np.asarray(b2, np.float32),
    )

    if _trace:
        _ensure_ntff_hook()
    in_maps = [
        {
            "x": np.ascontiguousarray(x[c * N_IMG : (c + 1) * N_IMG]),
            "w1p": w1p,
            "w2p": w2p,
            "aux": aux,
        }
        for c in range(N_CORES)
    ]
    res = run_bass_kernel_spmd(
        nc, in_maps, core_ids=list(range(N_CORES)), trace=_trace
    )
    out = np.concatenate([r["out"] for r in res.results], axis=0).astype(np.float32)
    if _trace:
        return out, res
    return out
